# revision 8
# baseline (speedup 1.0000x reference)
"""Trainium2 Bass kernel for the 4-layer LSTM (T=128, B=64, H=1024).

Strategy (v2): 8 active cores = 4 pipeline stages (layer j) x 2 batch
halves (32 each). Group A = cores 0-3 handles batch[0:32], group B =
cores 4-7 handles batch[32:64]; within a group, core j runs layer j.
Everything lives in transposed space (zT = [4H, B2]) so no per-step
transposes are needed.

Per tick (G=2 steps), each core:
  - AllGather (4-rank, within its group) moves the previous tick's
    c-chunks between stages; double-buffered, fully overlapped.
  - PE: hU(s0) -> xW-half-A(next chunk) -> hU(s1) -> xW-half-B. The
    x@W for chunk k+1 is accumulated into the *other* PSUM tile during
    the gate-latency bubbles of chunk k, so the PE never idles.
  - Act/DVE: LSTM gates for each step, overlapped with the xW matmuls.

Layer spacing D=4 ticks gives the collective + src DMA + xW each a full
tick of slack. States start at zero and stay exactly zero until real
data arrives, so no per-rank resets are needed.

Output = cell state of layer 3 at t=T-1 (reference returns bb[L-1]);
half A from core 3, half B from core 7.
"""

import sys

for p in ("/opt/trn_rl_repo",):
    if p not in sys.path:
        sys.path.insert(0, p)

import numpy as np
import ml_dtypes

T, B, H, L = 128, 64, 1024, 4
FH = 4 * H
KT = H // 128           # 8 K-tiles
MT = FH // 128          # 32 M-tiles
B2 = B // 2             # per-core batch half
G = 2                   # steps per chunk
GB = G * B2             # chunk free-dim (steps x half-batch) = 64
NCH = T // G            # chunks per layer
D = 4                   # tick spacing between layers
NTICKS = NCH + (L - 1) * D + 3   # stage j computes chunk k at tick k+D*j+3
N_CORES = 8
MMPB = 2048 // (GB * 4)  # mm blocks per 2KB PSUM bank

_CACHE = {}


def _build(nticks=NTICKS):
    import concourse.bacc as bacc
    import concourse.mybir as mybir
    import concourse.tile as tile

    bf16, f32, i32 = mybir.dt.bfloat16, mybir.dt.float32, mybir.dt.int32
    AF = mybir.ActivationFunctionType
    Alu = mybir.AluOpType

    nc = bacc.Bacc("TRN2", target_bir_lowering=False, debug=False,
                   num_devices=N_CORES)

    # weights in mm-major layout: col = mm*KT*128 + k*128 + m
    w_in = nc.dram_tensor("w_loc", [128, MT * KT * 128], bf16,
                          kind="ExternalInput")
    u_in = nc.dram_tensor("u_loc", [128, MT * KT * 128], bf16,
                          kind="ExternalInput")
    src_static = nc.dram_tensor("src_static", [H, T * B2], bf16,
                                kind="ExternalInput")
    rparam = nc.dram_tensor("rparam", [1, 2], i32, kind="ExternalInput")
    out_ext = nc.dram_tensor("out", [128, KT * B2], f32, kind="ExternalOutput")

    # DRAM bounce buffers (double-buffered by tick parity)
    c_out = [nc.dram_tensor(f"c_out{i}", [H, GB], bf16) for i in range(2)]
    gath = [nc.dram_tensor(f"gath{i}", [4, H, GB], bf16) for i in range(2)]

    with tile.TileContext(nc) as tc:
        with (
            tc.tile_pool(name="wp", bufs=1) as wp,
            tc.tile_pool(name="sp", bufs=1) as sp,
            tc.tile_pool(name="srcp", bufs=3) as srcp,
            tc.tile_pool(name="ewp", bufs=2) as ewp,
            tc.tile_pool(name="zp", bufs=2, space="PSUM") as zp,
        ):
            # ---- preamble -------------------------------------------------
            w_sb = wp.tile([128, MT * KT * 128], bf16)
            u_sb = wp.tile([128, MT * KT * 128], bf16)
            half = MT * KT * 128 // 2
            for i in range(2):
                nc.sync.dma_start(w_sb[:, i * half:(i + 1) * half],
                                  w_in[:, i * half:(i + 1) * half])
                nc.sync.dma_start(u_sb[:, i * half:(i + 1) * half],
                                  u_in[:, i * half:(i + 1) * half])

            rp_sb = sp.tile([1, 2], i32)
            nc.sync.dma_start(rp_sb[:], rparam[:])
            rv = nc.values_load(rp_sb[:1, 0:1].to_broadcast((1, 1)))

            zsb = sp.tile([128, KT * GB], bf16)
            nc.gpsimd.memset(zsb[:], 0.0)
            for i in range(2):
                nc.sync.dma_start(
                    c_out[i].rearrange("(k p) n -> p k n", p=128),
                    zsb[:].rearrange("p (k n) -> p k n", k=KT))
                for s in range(4):
                    nc.sync.dma_start(
                        gath[i][s].rearrange("(k p) n -> p k n", p=128),
                        zsb[:].rearrange("p (k n) -> p k n", k=KT))

            # state (double-buffered by step parity; parity 0 at tick start)
            cT = [sp.tile([128, KT * B2], f32, name=f"cT{i}") for i in range(2)]
            hT = [sp.tile([128, KT * B2], bf16, name=f"hT{i}") for i in range(2)]
            for i in range(2):
                nc.gpsimd.memset(cT[i][:], 0.0)
                nc.gpsimd.memset(hT[i][:], 0.0)

            src_tiles = {}
            psz_tiles = {}

            def wslice(wt, mm, k):
                base = (mm * KT + k) * 128
                return wt[:, base:base + 128]

            def emit_xw(psz, src_sb, mm_lo, mm_hi):
                for mm in range(mm_lo, mm_hi):
                    for k in range(KT):
                        nc.tensor.matmul(
                            psz[:, mm * GB:(mm + 1) * GB],
                            wslice(w_sb, mm, k),
                            src_sb[:, k * GB:(k + 1) * GB],
                            start=(mm % MMPB == 0 and k == 0), stop=False,
                            skip_group_check=True,
                        )

            # ---- tick loop ------------------------------------------------
            for tau in range(nticks):
                nc.gpsimd.collective_compute(
                    "AllGather", Alu.bypass,
                    replica_groups=[[0, 1, 2, 3], [4, 5, 6, 7]],
                    ins=[c_out[(tau - 1) % 2].ap().opt()],
                    outs=[gath[tau % 2].ap().opt()],
                )

                # source chunk DMA (for next tick's xW): rank0 from
                # src_static, ranks>=1 from gathered slot (rank-1).
                src_sb = srcp.tile([128, KT * GB], bf16, tag="src",
                                   name=f"src_{tau}")
                src_tiles[tau] = src_sb
                kchunk = min(max(tau - 1, 0), NCH - 1)
                with tc.If(rv == 0) as cmp:
                    nc.sync.dma_start(
                        src_sb[:].rearrange("p (k n) -> p k n", k=KT),
                        src_static[:, kchunk * GB:(kchunk + 1) * GB]
                        .rearrange("(k p) n -> p k n", p=128))
                with cmp.Else():
                    for j in range(1, 4):
                        with tc.If(rv == j):
                            nc.sync.dma_start(
                                src_sb[:].rearrange("p (k n) -> p k n", k=KT),
                                gath[(tau - 1) % 2][j - 1]
                                .rearrange("(k p) n -> p k n", p=128))

                # PSUM tile for the chunk whose xW we accumulate this tick
                psz_new = zp.tile([128, MT * GB], f32, tag="Z",
                                  name=f"psz_{tau}")
                psz_tiles[tau] = psz_new

                if tau < 3:
                    # warmup: xW only (no real chunk to consume yet)
                    if tau == 2:
                        emit_xw(psz_new, src_tiles[tau - 1], 0, MT)
                    continue

                psz = psz_tiles[tau - 1]   # z = xW(chunk) + hU, consumed now
                xw_src = src_tiles[tau - 1]

                for s in range(G):
                    h_prev = hT[s % 2]
                    c_prev = cT[s % 2]
                    h_new = hT[(s + 1) % 2]
                    c_new = cT[(s + 1) % 2]
                    # h @ U accumulated on top of x@W (start=False)
                    for mm in range(MT):
                        for k in range(KT):
                            nc.tensor.matmul(
                                psz[:, mm * GB + s * B2:
                                    mm * GB + (s + 1) * B2],
                                wslice(u_sb, mm, k),
                                h_prev[:, k * B2:(k + 1) * B2],
                                start=False,
                                stop=(s == G - 1 and mm % MMPB == MMPB - 1
                                      and k == KT - 1),
                                skip_group_check=True,
                            )

                    # gates: mm 0-7 = i, 8-15 = f, 16-23 = g, 24-31 = o
                    def zview(g0, g1, s=s, psz=psz):
                        return psz[:].rearrange(
                            "p (mm n) -> p mm n", n=GB
                        )[:, g0 * KT:g1 * KT, s * B2:(s + 1) * B2]
                    sif = ewp.tile([128, 2 * KT * B2], f32, tag="sif",
                                   name=f"sif_{tau}_{s}")
                    tg = ewp.tile([128, KT * B2], f32, tag="tg",
                                  name=f"tg_{tau}_{s}")
                    so = ewp.tile([128, KT * B2], f32, tag="so",
                                  name=f"so_{tau}_{s}")
                    nc.scalar.activation(
                        sif[:].rearrange("p (mm n) -> p mm n", n=B2),
                        zview(0, 2), AF.Sigmoid)
                    nc.scalar.activation(
                        tg[:].rearrange("p (mm n) -> p mm n", n=B2),
                        zview(2, 3), AF.Tanh)
                    nc.scalar.activation(
                        so[:].rearrange("p (mm n) -> p mm n", n=B2),
                        zview(3, 4), AF.Sigmoid)
                    fc = ewp.tile([128, KT * B2], f32, tag="fc",
                                  name=f"fc_{tau}_{s}")
                    ig = ewp.tile([128, KT * B2], f32, tag="ig",
                                  name=f"ig_{tau}_{s}")
                    nc.vector.tensor_tensor(fc[:], sif[:, KT * B2:],
                                            c_prev[:], Alu.mult)
                    nc.vector.tensor_tensor(ig[:], sif[:, 0:KT * B2], tg[:],
                                            Alu.mult)
                    nc.vector.tensor_tensor(c_new[:], fc[:], ig[:], Alu.add)
                    th = ewp.tile([128, KT * B2], f32, tag="th",
                                  name=f"th_{tau}_{s}")
                    nc.scalar.activation(th[:], c_new[:], AF.Tanh)
                    nc.vector.tensor_tensor(h_new[:], so[:], th[:], Alu.mult)
                    # cast c for transport
                    cbf = ewp.tile([128, KT * B2], bf16, tag="cbf",
                                   name=f"cbf_{tau}_{s}")
                    nc.vector.tensor_copy(cbf[:], c_new[:])
                    nc.sync.dma_start(
                        c_out[tau % 2][:, s * B2:(s + 1) * B2]
                        .rearrange("(k p) n -> p k n", p=128),
                        cbf[:].rearrange("p (k n) -> p k n", k=KT))

                    # xW for the next chunk, interleaved after each hU
                    emit_xw(psz_new, xw_src,
                            s * (MT // G), (s + 1) * (MT // G))

            # final state out (ranks 3 and 7 carry the answer)
            nc.sync.dma_start(out_ext[:], cT[0][:])
    nc.finalize()
    return nc


def _prep_in_maps(inputs, W, U, b):
    # transposed input, batch-halved: [H, T*B2] with chunk cols contiguous
    xT = inputs.astype(np.float32).transpose(2, 0, 1)  # [H, T, B]
    halves = [
        np.ascontiguousarray(xT[:, :, h * B2:(h + 1) * B2]
                             .reshape(H, T * B2)).astype(ml_dtypes.bfloat16)
        for h in range(2)
    ]
    zeros_src = np.zeros((H, T * B2), dtype=ml_dtypes.bfloat16)

    def mm_major(M):  # [H, FH] -> [128, MT*KT*128]
        return np.ascontiguousarray(
            M.reshape(KT, 128, MT, 128).transpose(1, 2, 0, 3)
            .reshape(128, MT * KT * 128)).astype(ml_dtypes.bfloat16)

    Wmm = [mm_major(W[j].astype(np.float32)) for j in range(L)]
    Umm = [mm_major(U[j].astype(np.float32)) for j in range(L)]
    in_maps = []
    for r in range(N_CORES):
        j = r % 4
        in_maps.append({
            "w_loc": Wmm[j],
            "u_loc": Umm[j],
            "src_static": halves[r // 4] if j == 0 else zeros_src,
            "rparam": np.array([[j, 0]], dtype=np.int32),
        })
    return in_maps


def kernel(inputs, W, U, b):
    assert not np.any(b), "nonzero bias not implemented"
    from concourse.bass_utils import run_bass_kernel_spmd

    if "nc" not in _CACHE:
        _CACHE["nc"] = _build()
    nc = _CACHE["nc"]
    in_maps = _prep_in_maps(inputs, W, U, b)
    res = run_bass_kernel_spmd(nc, in_maps, core_ids=list(range(N_CORES)))
    c = np.zeros((B, H), dtype=np.float32)
    for hi, r in ((0, 3), (1, 7)):
        ct = res.results[r]["out"]  # [128, KT*B2], k-tile k at cols k*B2
        for k in range(KT):
            c[hi * B2:(hi + 1) * B2, k * 128:(k + 1) * 128] = \
                ct[:, k * B2:(k + 1) * B2].T
    return c


# revision 16
# speedup vs baseline: 1.0439x; 1.0439x over previous
"""Trainium2 Bass kernel for the 4-layer LSTM (T=128, B=64, H=1024).

Strategy (v2): 8 active cores = 4 pipeline stages (layer j) x 2 batch
halves (32 each). Group A = cores 0-3 handles batch[0:32], group B =
cores 4-7 handles batch[32:64]; within a group, core j runs layer j.
Everything lives in transposed space (zT = [4H, B2]) so no per-step
transposes are needed.

Per tick (G=2 steps), each core:
  - AllGather (4-rank, within its group) moves the previous tick's
    c-chunks between stages; double-buffered, fully overlapped.
  - PE: hU(s0) -> xW-half-A(next chunk) -> hU(s1) -> xW-half-B. The
    x@W for chunk k+1 is accumulated into the *other* PSUM tile during
    the gate-latency bubbles of chunk k, so the PE never idles.
  - Act/DVE: LSTM gates for each step, overlapped with the xW matmuls.

Layer spacing D=4 ticks gives the collective + src DMA + xW each a full
tick of slack. States start at zero and stay exactly zero until real
data arrives, so no per-rank resets are needed.

Output = cell state of layer 3 at t=T-1 (reference returns bb[L-1]);
half A from core 3, half B from core 7.
"""

import sys

for p in ("/opt/trn_rl_repo",):
    if p not in sys.path:
        sys.path.insert(0, p)

import numpy as np
import ml_dtypes

T, B, H, L = 128, 64, 1024, 4
FH = 4 * H
KT = H // 128           # 8 K-tiles
MT = FH // 128          # 32 M-tiles
B2 = B // 2             # per-core batch half
G = 2                   # steps per chunk
GB = G * B2             # chunk free-dim (steps x half-batch) = 64
NCH = T // G            # chunks per layer
D = 3                   # tick spacing between layers
NTICKS = NCH + (L - 1) * D + 3   # stage j computes chunk k at tick k+D*j+3
N_CORES = 8
MMPB = 2048 // (GB * 4)  # mm blocks per 2KB PSUM bank

_CACHE = {}


def _build(nticks=NTICKS):
    import concourse.bacc as bacc
    import concourse.mybir as mybir
    import concourse.tile as tile

    bf16, f32, i32 = mybir.dt.bfloat16, mybir.dt.float32, mybir.dt.int32
    AF = mybir.ActivationFunctionType
    Alu = mybir.AluOpType

    nc = bacc.Bacc("TRN2", target_bir_lowering=False, debug=False,
                   num_devices=N_CORES)

    # weights in mm-major layout: col = mm*KT*128 + k*128 + m
    w_in = nc.dram_tensor("w_loc", [128, MT * KT * 128], bf16,
                          kind="ExternalInput")
    u_in = nc.dram_tensor("u_loc", [128, MT * KT * 128], bf16,
                          kind="ExternalInput")
    src_static = nc.dram_tensor("src_static", [H, T * B2], bf16,
                                kind="ExternalInput")
    rparam = nc.dram_tensor("rparam", [1, 2], i32, kind="ExternalInput")
    out_ext = nc.dram_tensor("out", [128, KT * B2], f32, kind="ExternalOutput")

    # DRAM bounce buffers (double-buffered by tick parity)
    c_out = [nc.dram_tensor(f"c_out{i}", [H, GB], bf16) for i in range(2)]
    gath = [nc.dram_tensor(f"gath{i}", [4, H, GB], bf16) for i in range(2)]

    with tile.TileContext(nc) as tc:
        with (
            tc.tile_pool(name="wp", bufs=1) as wp,
            tc.tile_pool(name="sp", bufs=1) as sp,
            tc.tile_pool(name="srcp", bufs=3) as srcp,
            tc.tile_pool(name="ewp", bufs=2) as ewp,
            tc.tile_pool(name="zp", bufs=2, space="PSUM") as zp,
        ):
            # ---- preamble -------------------------------------------------
            w_sb = wp.tile([128, MT * KT * 128], bf16)
            u_sb = wp.tile([128, MT * KT * 128], bf16)
            half = MT * KT * 128 // 2
            for i in range(2):
                nc.sync.dma_start(w_sb[:, i * half:(i + 1) * half],
                                  w_in[:, i * half:(i + 1) * half])
                nc.sync.dma_start(u_sb[:, i * half:(i + 1) * half],
                                  u_in[:, i * half:(i + 1) * half])

            rp_sb = sp.tile([1, 2], i32)
            nc.sync.dma_start(rp_sb[:], rparam[:])
            rv = nc.values_load(rp_sb[:1, 0:1].to_broadcast((1, 1)))

            zsb = sp.tile([128, KT * GB], bf16)
            nc.gpsimd.memset(zsb[:], 0.0)
            for i in range(2):
                nc.sync.dma_start(
                    c_out[i].rearrange("(k p) n -> p k n", p=128),
                    zsb[:].rearrange("p (k n) -> p k n", k=KT))
                for s in range(4):
                    nc.sync.dma_start(
                        gath[i][s].rearrange("(k p) n -> p k n", p=128),
                        zsb[:].rearrange("p (k n) -> p k n", k=KT))

            # state (double-buffered by step parity; parity 0 at tick start)
            cT = [sp.tile([128, KT * B2], f32, name=f"cT{i}") for i in range(2)]
            hT = [sp.tile([128, KT * B2], bf16, name=f"hT{i}") for i in range(2)]
            for i in range(2):
                nc.gpsimd.memset(cT[i][:], 0.0)
                nc.gpsimd.memset(hT[i][:], 0.0)

            src_tiles = {}
            psz_tiles = {}

            def wslice(wt, mm, k):
                base = (mm * KT + k) * 128
                return wt[:, base:base + 128]

            def emit_xw(psz, src_sb, mm_lo, mm_hi):
                for mm in range(mm_lo, mm_hi):
                    for k in range(KT):
                        nc.tensor.matmul(
                            psz[:, mm * GB:(mm + 1) * GB],
                            wslice(w_sb, mm, k),
                            src_sb[:, k * GB:(k + 1) * GB],
                            start=(mm % MMPB == 0 and k == 0), stop=False,
                            skip_group_check=True,
                        )

            # ---- tick loop ------------------------------------------------
            for tau in range(nticks):
                nc.gpsimd.collective_compute(
                    "AllGather", Alu.bypass,
                    replica_groups=[[0, 1, 2, 3], [4, 5, 6, 7]],
                    ins=[c_out[(tau - 1) % 2].ap().opt()],
                    outs=[gath[tau % 2].ap().opt()],
                )

                # source chunk DMA (for next tick's xW): rank0 from
                # src_static, ranks>=1 from gathered slot (rank-1).
                src_sb = srcp.tile([128, KT * GB], bf16, tag="src",
                                   name=f"src_{tau}")
                src_tiles[tau] = src_sb
                kchunk = min(max(tau - 2, 0), NCH - 1)
                with tc.If(rv == 0) as cmp:
                    nc.sync.dma_start(
                        src_sb[:].rearrange("p (k n) -> p k n", k=KT),
                        src_static[:, kchunk * GB:(kchunk + 1) * GB]
                        .rearrange("(k p) n -> p k n", p=128))
                with cmp.Else():
                    for j in range(1, 4):
                        with tc.If(rv == j):
                            nc.sync.dma_start(
                                src_sb[:].rearrange("p (k n) -> p k n", k=KT),
                                gath[(tau - 1) % 2][j - 1]
                                .rearrange("(k p) n -> p k n", p=128))

                # PSUM tile for the chunk whose xW we accumulate this tick
                psz_new = zp.tile([128, MT * GB], f32, tag="Z",
                                  name=f"psz_{tau}")
                psz_tiles[tau] = psz_new

                if tau < 3:
                    # warmup: xW only (no real chunk to consume yet)
                    if tau == 2:
                        emit_xw(psz_new, src_tiles[tau], 0, MT)
                    continue

                psz = psz_tiles[tau - 1]   # z = xW(chunk) + hU, consumed now
                xw_src = src_tiles[tau]    # src DMA'd this tick, xW'd this tick

                for s in range(G):
                    h_prev = hT[s % 2]
                    c_prev = cT[s % 2]
                    h_new = hT[(s + 1) % 2]
                    c_new = cT[(s + 1) % 2]
                    # h @ U accumulated on top of x@W (start=False)
                    for mm in range(MT):
                        for k in range(KT):
                            nc.tensor.matmul(
                                psz[:, mm * GB + s * B2:
                                    mm * GB + (s + 1) * B2],
                                wslice(u_sb, mm, k),
                                h_prev[:, k * B2:(k + 1) * B2],
                                start=False,
                                stop=(s == G - 1 and mm % MMPB == MMPB - 1
                                      and k == KT - 1),
                                skip_group_check=True,
                            )

                    # gates: mm 0-7 = i, 8-15 = f, 16-23 = g, 24-31 = o
                    def zview(g0, g1, s=s, psz=psz):
                        return psz[:].rearrange(
                            "p (mm n) -> p mm n", n=GB
                        )[:, g0 * KT:g1 * KT, s * B2:(s + 1) * B2]
                    sif = ewp.tile([128, 2 * KT * B2], f32, tag="sif",
                                   name=f"sif_{tau}_{s}")
                    tg = ewp.tile([128, KT * B2], f32, tag="tg",
                                  name=f"tg_{tau}_{s}")
                    so = ewp.tile([128, KT * B2], f32, tag="so",
                                  name=f"so_{tau}_{s}")
                    nc.scalar.activation(
                        sif[:].rearrange("p (mm n) -> p mm n", n=B2),
                        zview(0, 2), AF.Sigmoid)
                    nc.scalar.activation(
                        tg[:].rearrange("p (mm n) -> p mm n", n=B2),
                        zview(2, 3), AF.Tanh)
                    nc.scalar.activation(
                        so[:].rearrange("p (mm n) -> p mm n", n=B2),
                        zview(3, 4), AF.Sigmoid)
                    fc = ewp.tile([128, KT * B2], f32, tag="fc",
                                  name=f"fc_{tau}_{s}")
                    ig = ewp.tile([128, KT * B2], f32, tag="ig",
                                  name=f"ig_{tau}_{s}")
                    nc.vector.tensor_tensor(fc[:], sif[:, KT * B2:],
                                            c_prev[:], Alu.mult)
                    nc.vector.tensor_tensor(ig[:], sif[:, 0:KT * B2], tg[:],
                                            Alu.mult)
                    nc.vector.tensor_tensor(c_new[:], fc[:], ig[:], Alu.add)
                    th = ewp.tile([128, KT * B2], f32, tag="th",
                                  name=f"th_{tau}_{s}")
                    nc.scalar.activation(th[:], c_new[:], AF.Tanh)
                    nc.vector.tensor_tensor(h_new[:], so[:], th[:], Alu.mult)
                    # cast c for transport
                    cbf = ewp.tile([128, KT * B2], bf16, tag="cbf",
                                   name=f"cbf_{tau}_{s}")
                    nc.vector.tensor_copy(cbf[:], c_new[:])
                    nc.sync.dma_start(
                        c_out[tau % 2][:, s * B2:(s + 1) * B2]
                        .rearrange("(k p) n -> p k n", p=128),
                        cbf[:].rearrange("p (k n) -> p k n", k=KT))

                    # xW for the next chunk, interleaved after each hU
                    emit_xw(psz_new, xw_src,
                            s * (MT // G), (s + 1) * (MT // G))

            # final state out (ranks 3 and 7 carry the answer)
            nc.sync.dma_start(out_ext[:], cT[0][:])
    nc.finalize()
    return nc


def _prep_in_maps(inputs, W, U, b):
    # transposed input, batch-halved: [H, T*B2] with chunk cols contiguous
    xT = inputs.astype(np.float32).transpose(2, 0, 1)  # [H, T, B]
    halves = [
        np.ascontiguousarray(xT[:, :, h * B2:(h + 1) * B2]
                             .reshape(H, T * B2)).astype(ml_dtypes.bfloat16)
        for h in range(2)
    ]
    zeros_src = np.zeros((H, T * B2), dtype=ml_dtypes.bfloat16)

    def mm_major(M):  # [H, FH] -> [128, MT*KT*128]
        return np.ascontiguousarray(
            M.reshape(KT, 128, MT, 128).transpose(1, 2, 0, 3)
            .reshape(128, MT * KT * 128)).astype(ml_dtypes.bfloat16)

    Wmm = [mm_major(W[j].astype(np.float32)) for j in range(L)]
    Umm = [mm_major(U[j].astype(np.float32)) for j in range(L)]
    in_maps = []
    for r in range(N_CORES):
        j = r % 4
        in_maps.append({
            "w_loc": Wmm[j],
            "u_loc": Umm[j],
            "src_static": halves[r // 4] if j == 0 else zeros_src,
            "rparam": np.array([[j, 0]], dtype=np.int32),
        })
    return in_maps


def kernel(inputs, W, U, b):
    assert not np.any(b), "nonzero bias not implemented"
    from concourse.bass_utils import run_bass_kernel_spmd

    if "nc" not in _CACHE:
        _CACHE["nc"] = _build()
    nc = _CACHE["nc"]
    in_maps = _prep_in_maps(inputs, W, U, b)
    res = run_bass_kernel_spmd(nc, in_maps, core_ids=list(range(N_CORES)))
    c = np.zeros((B, H), dtype=np.float32)
    for hi, r in ((0, 3), (1, 7)):
        ct = res.results[r]["out"]  # [128, KT*B2], k-tile k at cols k*B2
        for k in range(KT):
            c[hi * B2:(hi + 1) * B2, k * 128:(k + 1) * 128] = \
                ct[:, k * B2:(k + 1) * B2].T
    return c
